# revision 1
# baseline (speedup 1.0000x reference)
"""LocallyConnected2d Trainium2 kernel.

y[b,o,h,w] = sum_{i,ky,kx} x[b,i,h+ky-1,w+kx-1] * weight[i,o,h,w,ky,kx] + bias[o,h,w]

Shapes: x [64,64,32,32], weight [64,64,32,32,3,3], bias [64,32,32] -> y [64,64,32,32].

Strategy
--------
Spatial sharding over H_out: 8 cores x 4 output rows each (receptive fields
need rows h-1..h+4 of x, packed per-core on host).

Per output location (h,w): a K=576 x M=64(cout) x N=64(batch) matmul,
executed as 5 PSUM-accumulating matmuls: 4 chunks of K=128 (each chunk = two
kernel offsets x 64 cin) + 1 tail chunk of K=64 (offset (2,2)).

A K=128 chunk spans two kernel offsets whose x data must appear at the SAME
free-dim offset on partitions 0-63 and 64-127. We pre-shift the bottom copy on
host: X1 has the bottom half shifted by 1 (serves pairs (ky,0)+(ky,1)), X34 is
shifted by 34 (serves pair (0,2)+(1,2)). Offsets are in units of 64-batch
blocks over the flattened (row, col) slab of the padded x slice.

All inputs are host-packed into exact per-core SBUF images so every DMA is a
plain contiguous [P, F] load.
"""

import sys

sys.path.insert(0, "/opt/trn_rl_repo")

import numpy as np

B, CIN, COUT, H, W = 64, 64, 64, 32, 32
K = 3
HOUT, WOUT = 32, 32
NCORES = 8
ROWS = HOUT // NCORES  # output rows per core
SLAB_R = ROWS + 2      # x rows needed per core (halo)
SLAB_C = W + 2         # padded width
RC = SLAB_R * SLAB_C   # flattened (row, col) length

# chunk pairing: j=0..3 -> (ky0,kx0)+(ky1,kx1); tail = (2,2)
PAIRS = [((0, 0), (0, 1)), ((1, 0), (1, 1)), ((2, 0), (2, 1)), ((0, 2), (1, 2))]
TAIL = (2, 2)

_nc_cache = {}


def _build_bass():
    import concourse.bass as bass
    import concourse.tile as tile
    from concourse import bacc, mybir

    f32 = mybir.dt.float32
    nc = bacc.Bacc(None, target_bir_lowering=False)

    x1_d = nc.dram_tensor("x1", (128, RC, B), f32, kind="ExternalInput")
    x34_d = nc.dram_tensor("x34", (128, RC, B), f32, kind="ExternalInput")
    wmain_d = nc.dram_tensor("wmain", (ROWS, 128, WOUT, 4, COUT), f32, kind="ExternalInput")
    wtail_d = nc.dram_tensor("wtail", (ROWS, 64, WOUT, COUT), f32, kind="ExternalInput")
    bias_d = nc.dram_tensor("bias", (ROWS, COUT, WOUT), f32, kind="ExternalInput")
    out_d = nc.dram_tensor("out", (ROWS, COUT, WOUT, B), f32, kind="ExternalOutput")

    with tile.TileContext(nc) as tc:
        with (
            tc.tile_pool(name="xpool", bufs=1) as xpool,
            tc.tile_pool(name="wpool", bufs=2) as wpool,
            tc.tile_pool(name="opool", bufs=2) as opool,
            tc.tile_pool(name="bpool", bufs=1) as bpool,
            tc.tile_pool(name="psum", bufs=8, space=bass.MemorySpace.PSUM) as psum,
        ):
            x1 = xpool.tile([128, RC, B], f32, tag="x1")
            x34 = xpool.tile([128, RC, B], f32, tag="x34")
            nc.sync.dma_start(x1[:], x1_d[:])
            nc.sync.dma_start(x34[:], x34_d[:])

            bi = bpool.tile([COUT, ROWS, WOUT], f32, tag="bias")
            nc.sync.dma_start(
                bi[:], bias_d.rearrange("h o w -> o h w")
            )

            for h in range(ROWS):
                wm = wpool.tile([128, WOUT, 4, COUT], f32, tag="wm")
                wt = wpool.tile([64, WOUT, COUT], f32, tag="wt")
                nc.sync.dma_start(wm[:], wmain_d[h])
                nc.sync.dma_start(wt[:], wtail_d[h])
                ot = opool.tile([COUT, WOUT, B], f32, tag="out")

                for w in range(WOUT):
                    ps = psum.tile([COUT, B], f32, tag="ps")
                    for j, ((ky0, kx0), _) in enumerate(PAIRS):
                        xsrc = x34 if j == 3 else x1
                        rc = (h + ky0) * SLAB_C + (w + kx0)
                        nc.tensor.matmul(
                            ps[:],
                            wm[:, w, j, :],
                            xsrc[:, rc, :],
                            start=(j == 0),
                            stop=False,
                        )
                    rc_t = (h + TAIL[0]) * SLAB_C + (w + TAIL[1])
                    nc.tensor.matmul(
                        ps[:],
                        wt[:, w, :],
                        x1[0:64, rc_t, :],
                        start=False,
                        stop=True,
                    )
                    nc.any.tensor_scalar_add(ot[:, w, :], ps[:], bi[:, h, w : w + 1])

                nc.sync.dma_start(out_d[h], ot[:])

    nc.compile()
    return nc


def get_nc():
    if "nc" not in _nc_cache:
        _nc_cache["nc"] = _build_bass()
    return _nc_cache["nc"]


def _shift(s, d):
    """s: [64, RC, B]; returns s advanced by d blocks along axis 1, zero-filled."""
    out = np.zeros_like(s)
    out[:, : RC - d, :] = s[:, d:, :]
    return out


def pack_inputs(x, weight, bias):
    """Returns list of per-core in_maps (numpy, C-contiguous)."""
    x = np.asarray(x, dtype=np.float32)
    weight = np.asarray(weight, dtype=np.float32)
    bias = np.asarray(bias, dtype=np.float32)

    # padded x: [B, CIN, H+2, W+2]
    xp = np.zeros((B, CIN, H + 2, W + 2), dtype=np.float32)
    xp[:, :, 1:-1, 1:-1] = x

    # weight -> [h, w, ky, kx, cin, cout]
    wt = np.ascontiguousarray(np.transpose(weight, (2, 3, 4, 5, 0, 1)))

    ky0s = np.array([p[0][0] for p in PAIRS])
    kx0s = np.array([p[0][1] for p in PAIRS])
    ky1s = np.array([p[1][0] for p in PAIRS])
    kx1s = np.array([p[1][1] for p in PAIRS])

    in_maps = []
    for c in range(NCORES):
        h0 = c * ROWS
        # x slab rows h0-1 .. h0+ROWS (SLAB_R rows of padded x)
        slab = xp[:, :, h0 : h0 + SLAB_R, :]  # [B, CIN, SLAB_R, SLAB_C]
        s = np.transpose(slab, (1, 2, 3, 0)).reshape(CIN, RC, B)  # [cin, rc, b]
        x1 = np.concatenate([s, _shift(s, 1)], axis=0)
        x34 = np.concatenate([s, _shift(s, 34)], axis=0)

        wh = wt[h0 : h0 + ROWS]  # [ROWS, w, ky, kx, cin, cout]
        top = wh[:, :, ky0s, kx0s]  # [ROWS, w, j, cin, cout]
        bot = wh[:, :, ky1s, kx1s]
        # -> [ROWS, cin, w, j, cout]
        top = np.transpose(top, (0, 3, 1, 2, 4))
        bot = np.transpose(bot, (0, 3, 1, 2, 4))
        wmain = np.concatenate([top, bot], axis=1)  # [ROWS, 128, w, j, cout]
        wtail = np.transpose(wh[:, :, TAIL[0], TAIL[1]], (0, 2, 1, 3))  # [ROWS, cin, w, cout]

        bi = np.transpose(bias[:, h0 : h0 + ROWS, :], (1, 0, 2))  # [ROWS, cout, w]

        in_maps.append(
            {
                "x1": np.ascontiguousarray(x1),
                "x34": np.ascontiguousarray(x34),
                "wmain": np.ascontiguousarray(wmain),
                "wtail": np.ascontiguousarray(wtail),
                "bias": np.ascontiguousarray(bi),
            }
        )
    return in_maps


def unpack_outputs(results):
    """results: list of per-core out_maps with 'out' [ROWS, COUT, WOUT, B]."""
    full = np.concatenate([np.asarray(r["out"]) for r in results], axis=0)
    # [HOUT, COUT, WOUT, B] -> [B, COUT, HOUT, WOUT]
    return np.ascontiguousarray(np.transpose(full, (3, 1, 0, 2)))


def run(in_maps, **kwargs):
    from concourse import bass_utils

    nc = get_nc()
    return bass_utils.run_bass_kernel_spmd(
        nc, in_maps, core_ids=list(range(NCORES)), **kwargs
    )


def kernel(x, weight, bias):
    in_maps = pack_inputs(x, weight, bias)
    res = run(in_maps)
    return unpack_outputs(res.results)


if __name__ == "__main__":
    rng = np.random.default_rng(0)
    x = rng.standard_normal((B, CIN, H, W), dtype=np.float32)
    weight = rng.standard_normal((CIN, COUT, HOUT, WOUT, K, K), dtype=np.float32)
    bias = rng.standard_normal((COUT, HOUT, WOUT), dtype=np.float32)
    y = kernel(x, weight, bias)
    print("out", y.shape, y.dtype)



# revision 2
# speedup vs baseline: 3.6823x; 3.6823x over previous
"""LocallyConnected2d Trainium2 kernel (fp8e3 weights, paired-column matmuls).

y[b,o,h,w] = sum_{i,ky,kx} x[b,i,h+ky-1,w+kx-1] * weight[i,o,h,w,ky,kx] + bias[o,h,w]

Shapes: x [64,64,32,32], weight [64,64,32,32,3,3], bias [64,32,32] -> y [64,64,32,32].

Strategy
--------
Spatial sharding over H_out: 8 cores x 4 output rows each.

Per core, output columns are processed in PAIRS (2wp, 2wp+1) so each matmul has
M=128 stationary columns (cout 64+64 for the two locations) -> triggers the
compiler's Fast Weight Load (4x for fp8). The contraction K=128 stacks TWO
x-column-slabs (cin=64 each): adjacent locations share shifted receptive
fields, so slab xp[:, r, c] serves loc 2wp at dx=c-2wp and loc 2wp+1 at
dx=c-2wp-1. Per (pair, dy) two matmuls cover all six (loc, dx) blocks with 2
of 8 64x64 weight blocks zero (shipped as zeros).

6 accumulating matmuls per pair -> PSUM [128, 64] = y for both locations, then
a per-partition bias add (ACT/DVE) writes bf16 to SBUF, DMA'd out per h-row.

Precision: weights are e3m4 at scale 2 (x pre-scaled by 0.5 on host, exact),
x/out bf16. Measured rel err vs fp32 reference: ~1.34e-2 (gate 2e-2).

All packing/unpacking happens on host (not counted in HW exec time).
"""

import sys

sys.path.insert(0, "/opt/trn_rl_repo")

import ml_dtypes
import numpy as np

B, CIN, COUT, H, W = 64, 64, 64, 32, 32
K = 3
HOUT, WOUT = 32, 32
NCORES = 8
ROWS = HOUT // NCORES  # output rows per core
NPAIR = WOUT // 2      # column pairs per row
SLAB_R = ROWS + 2      # padded x rows needed per core
NO = W // 2 + 1        # column-slab pairs (o indexes cols (2o, 2o+1)), 17

_nc_cache = {}


def _build_bass():
    import concourse.bass as bass
    import concourse.tile as tile
    from concourse import bacc, mybir

    f32 = mybir.dt.float32
    bf16 = mybir.dt.bfloat16
    f8 = mybir.dt.float8e3
    nc = bacc.Bacc(None, target_bir_lowering=False)

    xa_d = nc.dram_tensor("xa", (128, SLAB_R, NO, B), bf16, kind="ExternalInput")
    wt_d = nc.dram_tensor(
        "wt", (128, ROWS, NPAIR, 3, 2, 128), f8, kind="ExternalInput"
    )
    bias_d = nc.dram_tensor("bias", (128, ROWS, NPAIR), f32, kind="ExternalInput")
    out_d = nc.dram_tensor("out", (ROWS, 128, NPAIR, B), bf16, kind="ExternalOutput")

    with tile.TileContext(nc) as tc:
        with (
            tc.tile_pool(name="xpool", bufs=1) as xpool,
            tc.tile_pool(name="wpool", bufs=1) as wpool,
            tc.tile_pool(name="opool", bufs=2) as opool,
            tc.tile_pool(name="bpool", bufs=1) as bpool,
            tc.tile_pool(name="psum", bufs=8, space=bass.MemorySpace.PSUM) as psum,
        ):
            xa = xpool.tile([128, SLAB_R, NO, B], bf16, tag="xa")
            nc.sync.dma_start(xa[:], xa_d[:])
            bi = bpool.tile([128, ROWS, NPAIR], f32, tag="bias")
            nc.sync.dma_start(bi[:], bias_d[:])

            wts = []
            for h in range(ROWS):
                wt = wpool.tile([128, NPAIR, 3, 2, 128], f8, tag=f"wt{h}")
                nc.sync.dma_start(wt[:], wt_d[:, h])
                wts.append(wt)

            for h in range(ROWS):
                ot = opool.tile([128, NPAIR, B], bf16, tag="out")
                for wp in range(NPAIR):
                    ps = psum.tile([128, B], f32, tag="ps")
                    k = 0
                    for dy in range(3):
                        for m in range(2):
                            nc.tensor.matmul(
                                ps[:],
                                wts[h][:, wp, dy, m, :],
                                xa[:, h + dy, wp + m, :],
                                start=(k == 0),
                                stop=(k == 5),
                            )
                            k += 1
                    nc.any.tensor_scalar_add(
                        ot[:, wp, :], ps[:], bi[:, h, wp : wp + 1]
                    )
                nc.sync.dma_start(out_d[h], ot[:])

    nc.compile()
    return nc


def get_nc():
    if "nc" not in _nc_cache:
        _nc_cache["nc"] = _build_bass()
    return _nc_cache["nc"]


def pack_inputs(x, weight, bias):
    """Returns list of per-core in_maps (numpy, C-contiguous)."""
    x = np.asarray(x, dtype=np.float32)
    weight = np.asarray(weight, dtype=np.float32)
    bias = np.asarray(bias, dtype=np.float32)

    # padded, pre-scaled x: [B, CIN, H+2, W+2] bf16 (scale 0.5 is exact)
    xp = np.zeros((B, CIN, H + 2, W + 2), dtype=np.float32)
    xp[:, :, 1:-1, 1:-1] = x * 0.5
    xp = xp.astype(ml_dtypes.bfloat16)

    # weights at scale 2, e3m4 (max normal +-15.5)
    wq = np.clip(weight * 2.0, -15.5, 15.5).astype(ml_dtypes.float8_e3m4)
    wt6 = np.transpose(wq, (2, 3, 4, 5, 0, 1))  # [h, w, dy, dx, cin, cout]
    A = wt6[:, 0::2]  # [h, wp, dy, dx, cin, cout]  (even locations)
    Bw = wt6[:, 1::2]  # (odd locations)

    # stationary tiles [h, wp, dy, m, p(K), col(M)]
    WT = np.zeros((HOUT, NPAIR, 3, 2, 128, 128), dtype=ml_dtypes.float8_e3m4)
    WT[:, :, :, 0, 0:64, 0:64] = A[:, :, :, 0]
    WT[:, :, :, 0, 64:128, 0:64] = A[:, :, :, 1]
    WT[:, :, :, 0, 64:128, 64:128] = Bw[:, :, :, 0]
    WT[:, :, :, 1, 0:64, 0:64] = A[:, :, :, 2]
    WT[:, :, :, 1, 0:64, 64:128] = Bw[:, :, :, 1]
    WT[:, :, :, 1, 64:128, 64:128] = Bw[:, :, :, 2]

    in_maps = []
    for c in range(NCORES):
        r0 = c * ROWS
        xe = xp[:, :, r0 : r0 + SLAB_R, 0::2]  # [B, cin, 6, 17]
        xo = xp[:, :, r0 : r0 + SLAB_R, 1::2]
        xa = np.concatenate(
            [np.transpose(xe, (1, 2, 3, 0)), np.transpose(xo, (1, 2, 3, 0))], axis=0
        )  # [128, 6, 17, B]

        wtc = np.transpose(WT[r0 : r0 + ROWS], (4, 0, 1, 2, 3, 5))
        # [128, ROWS, NPAIR, 3, 2, 128]

        bic = np.concatenate(
            [bias[:, r0 : r0 + ROWS, 0::2], bias[:, r0 : r0 + ROWS, 1::2]], axis=0
        )  # [128, ROWS, NPAIR]

        in_maps.append(
            {
                "xa": np.ascontiguousarray(xa),
                "wt": np.ascontiguousarray(wtc),
                "bias": np.ascontiguousarray(bic),
            }
        )
    return in_maps


def unpack_outputs(results):
    """results: per-core out_maps with 'out' [ROWS, 128, NPAIR, B] bf16."""
    full = np.stack([np.asarray(r["out"]) for r in results]).astype(np.float32)
    # [8, ROWS, 128, NPAIR, B]
    y = np.empty((B, COUT, HOUT, WOUT), dtype=np.float32)
    even = full[:, :, 0:64]  # [core, h, cout, wp, b]
    odd = full[:, :, 64:128]
    y[:, :, :, 0::2] = np.transpose(even, (4, 2, 0, 1, 3)).reshape(
        B, COUT, HOUT, NPAIR
    )
    y[:, :, :, 1::2] = np.transpose(odd, (4, 2, 0, 1, 3)).reshape(B, COUT, HOUT, NPAIR)
    return y


def run(in_maps, **kwargs):
    from concourse import bass_utils

    nc = get_nc()
    return bass_utils.run_bass_kernel_spmd(
        nc, in_maps, core_ids=list(range(NCORES)), **kwargs
    )


def kernel(x, weight, bias):
    in_maps = pack_inputs(x, weight, bias)
    res = run(in_maps)
    return unpack_outputs(res.results)


if __name__ == "__main__":
    rng = np.random.default_rng(0)
    x = rng.standard_normal((B, CIN, H, W), dtype=np.float32)
    weight = rng.standard_normal((CIN, COUT, HOUT, WOUT, K, K), dtype=np.float32)
    bias = rng.standard_normal((COUT, HOUT, WOUT), dtype=np.float32)
    y = kernel(x, weight, bias)
    print("out", y.shape, y.dtype)
